# revision 1
# baseline (speedup 1.0000x reference)
"""Trainium2 Bass kernel for nn_JanusModel (sparse_attention, GQA, two mask groups).

Sharding: core c in [0,8) handles batch b=c//4 and query-row block q0=(c%4)*512.
Each core computes all 16 heads for its 512 query rows -> disjoint output slices,
no collectives. All heavy operands are laid out on host (transposes/permutes only).

On-device math per core (ARCH-T, scores kept transposed [sk, sq]):
  qT/kT/v projections (fp32r matmuls), scores.T = K @ qT/8 (row-tiled head pairs),
  P = exp(scores) * exp(maskT) (ACT exp + DVE bf16 mul), AV col-tiled head pairs,
  rowsums via M=1 quad matmuls with a ones vector, divide, output projection.
"""

import os
import sys

import numpy as np

for _p in ("/opt/trn_rl_repo",):
    if os.path.isdir(_p) and _p not in sys.path:
        sys.path.insert(0, _p)

import concourse.bass as bass
import concourse.tile as tile
from concourse import bacc, mybir
from concourse.bass_utils import run_bass_kernel_spmd

B, S, D = 2, 2048, 1024
H, KVH, HD = 16, 4, 64
NCORES = 8
SQ = S // 4  # 512 query rows per core
P = 128
NKT = S // P  # 16 key tiles

# Head pairs: (a, b) share a kT tile; a uses kv head 2*(j//4), b uses +1.
PAIRS = [(0, 4), (1, 5), (2, 6), (3, 7), (8, 12), (9, 13), (10, 14), (11, 15)]

f32 = mybir.dt.float32
bf16 = mybir.dt.bfloat16
f32r = mybir.dt.float32r
EXP = mybir.ActivationFunctionType.Exp
DIV = mybir.AluOpType.divide

_CACHE = {}


def _r(ap):
    return ap.bitcast(f32r)


def _body(tc, xT, wqT, wkT, wvT, woT, mT, out):
    nc = tc.nc
    rs_dram = nc.dram_tensor("rs_scratch", [8, 2, SQ], f32).ap()
    xT_r = xT.rearrange("(c p) s -> c p s", p=P)        # [8,128,2048]
    wqT_r = wqT.rearrange("(c p) f -> c p f", p=P)      # [8,128,1024]
    wkT_r = wkT.rearrange("(c p) f -> c p f", p=P)      # [8,128,256]
    wvT_r = wvT.rearrange("(c p) f -> c p f", p=P)      # [8,128,256]
    woT_r = woT.rearrange("(c p) d -> c p d", p=P)      # [8,128,1024]
    mT_r = mT.rearrange("m (c p) q -> m p c q", p=P)    # [2,128,16,512]
    out_r = out.rearrange("(t p) d -> t p d", p=P)      # [4,128,1024]

    persist = tc.alloc_tile_pool(name="persist", bufs=1)
    qT_sb = persist.tile([P, 8, SQ], f32r, name="qT_sb")      # pair j: a rows 0:64, b rows 64:128
    kT_sb = persist.tile([P, 2, S], f32r, name="kT_sb")       # tile jt: kv 2jt rows 0:64, kv 2jt+1 rows 64:128
    v_sb = persist.tile([P, NKT, KVH * HD], bf16, name="v_sb")

    # ---------------- phase A: load x/w, projections ----------------
    with tc.tile_pool(name="xw", bufs=1) as xw, \
         tc.tile_pool(name="pps", bufs=4, space="PSUM") as pps:
        x_sb = xw.tile([P, 8, S], f32r, name="x_sb")
        wq_sb = xw.tile([P, 8, H * HD], f32r, name="wq_sb")
        wk_sb = xw.tile([P, 8, KVH * HD], f32r, name="wk_sb")
        wv_sb = xw.tile([P, 8, KVH * HD], f32r, name="wv_sb")
        for c in range(8):
            nc.gpsimd.dma_start(out=x_sb[:, c, :], in_=xT_r[c])
            nc.gpsimd.dma_start(out=wq_sb[:, c, :], in_=wqT_r[c])
            nc.gpsimd.dma_start(out=wk_sb[:, c, :], in_=wkT_r[c])
            nc.gpsimd.dma_start(out=wv_sb[:, c, :], in_=wvT_r[c])

        # q projection: out [128 qfeat(pair j), 512]; fold 1/sqrt(HD)=1/8 scale
        for j in range(8):
            ps = pps.tile([P, SQ], f32, tag="pq", name=f"psq{j}")
            for kc in range(8):
                nc.tensor.matmul(
                    ps, lhsT=_r(wq_sb[:, kc, j * P:(j + 1) * P]),
                    rhs=_r(x_sb[:, kc, 0:SQ]),
                    start=(kc == 0), stop=(kc == 7))
            nc.vector.tensor_scalar_mul(qT_sb[:, j, :], ps, 0.125)

        # k projection: kT tiles [128 kvfeat, 2048]
        for jt in range(2):
            for ns in range(4):
                ps = pps.tile([P, SQ], f32, tag="pq", name=f"psk{jt}{ns}")
                for kc in range(8):
                    nc.tensor.matmul(
                        ps, lhsT=_r(wk_sb[:, kc, jt * P:(jt + 1) * P]),
                        rhs=_r(x_sb[:, kc, ns * SQ:(ns + 1) * SQ]),
                        start=(kc == 0), stop=(kc == 7))
                nc.vector.tensor_copy(out=kT_sb[:, jt, ns * SQ:(ns + 1) * SQ], in_=ps)

        # v projection: natural [sk 128-tile, 256] -> bf16
        for t in range(NKT):
            ps = pps.tile([P, KVH * HD], f32, tag="pv", name=f"psv{t}")
            for kc in range(8):
                nc.tensor.matmul(
                    ps, lhsT=_r(x_sb[:, kc, t * P:(t + 1) * P]),
                    rhs=_r(wv_sb[:, kc, :]),
                    start=(kc == 0), stop=(kc == 7))
            nc.vector.tensor_copy(out=v_sb[:, t, :], in_=ps)

    # ---------------- phase B: masks exp, attention ----------------
    with tc.tile_pool(name="attn_sb", bufs=1) as asb:
        expm_sb = asb.tile([P, 2, NKT, SQ], bf16, name="expm_sb")
        attnT_sb = asb.tile([P, 8, SQ], f32r, name="attnT_sb")
        ones_bf = asb.tile([P, 1], bf16, name="ones_bf")
        nc.vector.memset(ones_bf, 1.0)

        with tc.tile_pool(name="ml", bufs=2) as mlp:
            for m in range(2):
                for tg in range(8):
                    ml = mlp.tile([P, 2, SQ], f32, tag="ml", name=f"ml{m}{tg}")
                    nc.sync.dma_start(out=ml, in_=mT_r[m, :, 2 * tg:2 * tg + 2, :])
                    nc.scalar.activation(
                        out=expm_sb[:, m, 2 * tg:2 * tg + 2, :], in_=ml, func=EXP)

        with tc.tile_pool(name="psA", bufs=1, space="PSUM") as psA, \
             tc.tile_pool(name="psB", bufs=1, space="PSUM") as psB, \
             tc.tile_pool(name="avp", bufs=1, space="PSUM") as avp, \
             tc.tile_pool(name="qdp", bufs=1, space="PSUM") as qdp, \
             tc.tile_pool(name="praw", bufs=3) as praw, \
             tc.tile_pool(name="ppool", bufs=4) as ppool, \
             tc.tile_pool(name="small", bufs=2) as small:
            quad = None
            for j, (ha, hb) in enumerate(PAIRS):
                jt = j // 4          # kT tile index
                m = j // 4           # mask index
                vca = (j // 4) * 2 * HD   # v column of kv head for a
                vcb = vca + HD
                if j % 2 == 0:
                    quad = qdp.tile([P, SQ], f32, tag="quad", name=f"quad{j}")
                ca = 64 * (j % 2)    # quad col for head a
                cb = ca + 32
                av = avp.tile([P, SQ], f32, tag="av", name=f"av{j}")
                pa_tiles, pb_tiles = [], []
                for g in range(6):
                    nt = min(3, NKT - 3 * g)
                    sA = psA.tile([P, 3, SQ], f32, tag="sA", name=f"sA{j}_{g}")
                    sB = psB.tile([P, 3, SQ], f32, tag="sB", name=f"sB{j}_{g}")
                    for i in range(nt):
                        t = 3 * g + i
                        nc.tensor.matmul(
                            sA[:, i, :], lhsT=_r(kT_sb[0:64, jt, t * P:(t + 1) * P]),
                            rhs=_r(qT_sb[0:64, j, :]), start=True, stop=True)
                        nc.tensor.matmul(
                            sB[:, i, :], lhsT=_r(kT_sb[64:128, jt, t * P:(t + 1) * P]),
                            rhs=_r(qT_sb[64:128, j, :]), start=True, stop=True)
                    prA = praw.tile([P, 3, SQ], bf16, tag="prA", name=f"prA{j}_{g}")
                    prB = praw.tile([P, 3, SQ], bf16, tag="prB", name=f"prB{j}_{g}")
                    nc.scalar.activation(out=prA[:, 0:nt, :], in_=sA[:, 0:nt, :], func=EXP)
                    nc.scalar.activation(out=prB[:, 0:nt, :], in_=sB[:, 0:nt, :], func=EXP)
                    pA = ppool.tile([P, 3, SQ], bf16, tag="pA", name=f"pA{j}_{g}")
                    pB = ppool.tile([P, 3, SQ], bf16, tag="pB", name=f"pB{j}_{g}")
                    nc.vector.tensor_mul(pA[:, 0:nt, :], prA[:, 0:nt, :],
                                         expm_sb[:, m, 3 * g:3 * g + nt, :])
                    nc.vector.tensor_mul(pB[:, 0:nt, :], prB[:, 0:nt, :],
                                         expm_sb[:, m, 3 * g:3 * g + nt, :])
                    pa_tiles.append(pA)
                    pb_tiles.append(pB)
                    # AV + rowsum consume this group's P tiles immediately
                    for i in range(nt):
                        t = 3 * g + i
                        st = (t == 0)
                        sp = (t == NKT - 1)
                        nc.tensor.matmul(av[0:64, :], lhsT=v_sb[:, t, vca:vca + HD],
                                         rhs=pA[:, i, :], start=st, stop=sp)
                        nc.tensor.matmul(av[64:128, :], lhsT=v_sb[:, t, vcb:vcb + HD],
                                         rhs=pB[:, i, :], start=st, stop=sp)
                        nc.tensor.matmul(quad[ca:ca + 1, :], lhsT=ones_bf[:, 0:1],
                                         rhs=pA[:, i, :], start=st, stop=sp,
                                         tile_position=(0, ca))
                        nc.tensor.matmul(quad[cb:cb + 1, :], lhsT=ones_bf[:, 0:1],
                                         rhs=pB[:, i, :], start=st, stop=sp,
                                         tile_position=(0, cb))
                # rowsums -> broadcast [128,512]; attnT = av / rs
                rs = small.tile([P, SQ], f32, tag="rs", name=f"rs{j}")
                nc.vector.tensor_copy(out=rs[ca:ca + 1, :], in_=quad[ca:ca + 1, :])
                nc.vector.tensor_copy(out=rs[cb:cb + 1, :], in_=quad[cb:cb + 1, :])
                nc.sync.dma_start(out=rs_dram[j, 0, :], in_=rs[ca:ca + 1, :])
                nc.sync.dma_start(out=rs_dram[j, 1, :], in_=rs[cb:cb + 1, :])
                bc = small.tile([P, SQ], f32, tag="bc", name=f"bc{j}")
                for half in range(2):
                    row = rs_dram[j, half, :]
                    bcast = bass.AP(tensor=row.tensor, offset=row.offset,
                                    ap=[[0, 64]] + list(row.ap))
                    nc.sync.dma_start(out=bc[64 * half:64 * half + 64, :], in_=bcast)
                nc.vector.reciprocal(out=bc, in_=bc)
                nc.vector.tensor_mul(attnT_sb[:, j, :], av, bc)

        # ---------------- phase C: output projection ----------------
        with tc.tile_pool(name="wo", bufs=2) as wop, \
             tc.tile_pool(name="ops", bufs=8, space="PSUM") as ops, \
             tc.tile_pool(name="osb", bufs=2) as osb:
            pso = [ops.tile([P, SQ], f32, tag="ops", name=f"pso{i}") for i in range(8)]
            for j in range(8):
                wo_sb = wop.tile([P, D], f32r, tag="wo", name=f"wo{j}")
                nc.gpsimd.dma_start(out=wo_sb, in_=woT_r[j])
                for st in range(4):
                    for nt in range(2):
                        nc.tensor.matmul(
                            pso[st * 2 + nt],
                            lhsT=_r(attnT_sb[:, j, st * P:(st + 1) * P]),
                            rhs=_r(wo_sb[:, nt * SQ:(nt + 1) * SQ]),
                            start=(j == 0), stop=(j == 7))
            for st in range(4):
                ob = osb.tile([P, D], f32, tag="ob", name=f"ob{st}")
                nc.vector.tensor_copy(out=ob[:, 0:SQ], in_=pso[st * 2])
                nc.vector.tensor_copy(out=ob[:, SQ:D], in_=pso[st * 2 + 1])
                nc.sync.dma_start(out=out_r[st], in_=ob)
    persist.release()


def _build():
    if "nc" in _CACHE:
        return _CACHE["nc"]
    nc = bacc.Bacc("TRN2", target_bir_lowering=False, debug=False)
    xT = nc.dram_tensor("xT", [D, S], f32, kind="ExternalInput").ap()
    wqT = nc.dram_tensor("wqT", [D, H * HD], f32, kind="ExternalInput").ap()
    wkT = nc.dram_tensor("wkT", [D, KVH * HD], f32, kind="ExternalInput").ap()
    wvT = nc.dram_tensor("wvT", [D, KVH * HD], f32, kind="ExternalInput").ap()
    woT = nc.dram_tensor("woT", [H * HD, D], f32, kind="ExternalInput").ap()
    mT = nc.dram_tensor("mT", [2, S, SQ], f32, kind="ExternalInput").ap()
    out = nc.dram_tensor("out", [SQ, D], f32, kind="ExternalOutput").ap()
    with tile.TileContext(nc) as tc:
        _body(tc, xT, wqT, wkT, wvT, woT, mT, out)
    nc.compile()
    _CACHE["nc"] = nc
    return nc


def _host_prep(hidden_states, full_mask, tag_mask, wq, wk, wv, wo):
    # pair-ordered feature permutation for wq columns / wo.T rows
    perm = np.concatenate([np.r_[a * HD:(a + 1) * HD, b * HD:(b + 1) * HD]
                           for a, b in PAIRS])
    wqT = np.ascontiguousarray(wq.T[:, perm], np.float32)      # [D, 1024]
    wkT = np.ascontiguousarray(wk.T, np.float32)               # [D, 256]
    wvT = np.ascontiguousarray(wv.T, np.float32)               # [D, 256]
    woT = np.ascontiguousarray(wo.T[perm, :], np.float32)      # [1024, D]
    masksT = [np.ascontiguousarray(full_mask[b, 0].T) for b in range(B)] + \
             [np.ascontiguousarray(tag_mask[b, 0].T) for b in range(B)]
    xTs = [np.ascontiguousarray(hidden_states[b].T, np.float32) for b in range(B)]
    in_maps = []
    for c in range(NCORES):
        b, q0 = c // 4, (c % 4) * SQ
        xT_c = np.roll(xTs[b], -q0, axis=1)
        fmT = np.roll(masksT[b][:, q0:q0 + SQ], -q0, axis=0)
        tgT = np.roll(masksT[2 + b][:, q0:q0 + SQ], -q0, axis=0)
        mT_c = np.ascontiguousarray(np.stack([fmT, tgT]), np.float32)
        in_maps.append({"xT": np.ascontiguousarray(xT_c), "wqT": wqT, "wkT": wkT,
                        "wvT": wvT, "woT": woT, "mT": mT_c})
    return in_maps


def kernel(hidden_states, full_mask, tag_mask, wq, wk, wv, wo, _trace=False):
    args = [np.asarray(a, np.float32) for a in
            (hidden_states, full_mask, tag_mask, wq, wk, wv, wo)]
    nc = _build()
    in_maps = _host_prep(*args)
    try:
        res = run_bass_kernel_spmd(nc, in_maps, core_ids=list(range(NCORES)),
                                   trace=_trace)
    except ModuleNotFoundError:
        res = run_bass_kernel_spmd(nc, in_maps, core_ids=list(range(NCORES)))
    _CACHE["last_results"] = res
    full = np.empty((B, S, D), np.float32)
    for c in range(NCORES):
        b, q0 = c // 4, (c % 4) * SQ
        full[b, q0:q0 + SQ, :] = res.results[c]["out"]
    return full

